# revision 7
# baseline (speedup 1.0000x reference)
"""Gaussian-kernel layer (exp(-||x - w_m||^2) + b_m) as a Bass/Tile TRN2 kernel.

Math (per row n of x, per center m):
    out[n, m] = exp(-(x2[n] + w2[m] - 2*x.w)) + b[m]
              = exp(2*(xw[n,m] - w2[m]/2 - x2[n]/2)) + b[m]

v2.5 mapping (vs the v1 baseline that ran the whole main loop at the
cold 1.2 GHz HAM clock):
  - x is cast to bf16 on the host and loaded ONLY via XBAR DMA-transpose
    -> x_t tiles [C, n].  No PE transpose-mode instructions (those kept
    the HAM clock-gate shut in v1), no natural-layout copy.
  - x2 row-sums come from the transposed tiles: xt2 = x_t*x_t (DVE),
    then a PE matmul with a [-0.5] column vector gives -x2/2 as a [1, n]
    row, which GPSIMD copies into row 1 of a persistent [2, ROWS]
    stationary tile (row 0 = ones).
  - K=2 preload: one matmul with lhsT=[ones; -x2/2] rhs=[-w2/2; ones]
    initializes PSUM with BOTH -w2[m]/2 and -x2[n]/2 in a single
    512-column stream, then the main bf16 matmul accumulates x.w.
  - ACT: bias-free exp(2*psum) over 3 PSUM banks per instruction.
    The exp argument is the complete -d2 <= 0, so no overflow.
  - DVE: 3-tile-wide bf16 adds (+b), 2x packed mode.
  - Output stored as bf16 (rel tolerance is 2e-2; exp(-d2) <= 3e-44 on
    this distribution so the output is b to within fp32 epsilon and
    bf16 rounding is ~4e-3), widened to fp32 on the host.
"""

from contextlib import ExitStack

import numpy as np
import ml_dtypes

import concourse.bacc as bacc
import concourse.bass as bass
import concourse.mybir as mybir
import concourse.tile as tile
from concourse.bass_utils import run_bass_kernel_spmd

B, H, W_, C, M = 16, 48, 48, 128, 512
N_CORES = 8
B_PER = B // N_CORES          # 2 batches per core
ROWS = B_PER * H * W_         # 4608 rows per core
P = 128                       # partition / row-tile size
N_TILES = ROWS // P           # 36
G = 3                         # tiles per group
N_G = N_TILES // G            # 12
GR = G * P                    # rows per group (384)

F32 = mybir.dt.float32
BF16 = mybir.dt.bfloat16

_NC_CACHE = {}


def _build_nc():
    nc = bacc.Bacc(
        "TRN2",
        target_bir_lowering=False,
        debug=False,
        num_devices=N_CORES,
    )
    x_d = nc.declare_dram_parameter("x", [ROWS, C], BF16, isOutput=False)
    w_d = nc.declare_dram_parameter("w", [C, M], F32, isOutput=False)
    b_d = nc.declare_dram_parameter("b", [1, M], F32, isOutput=False)
    o_d = nc.declare_dram_parameter("out", [ROWS, M], BF16, isOutput=True)

    AF = mybir.ActivationFunctionType

    with tile.TileContext(nc) as tc, ExitStack() as ctx:
        consts = ctx.enter_context(tc.tile_pool(name="consts", bufs=1))
        xt_pool = ctx.enter_context(tc.tile_pool(name="xt", bufs=N_G))
        x2_pool = ctx.enter_context(tc.tile_pool(name="x2", bufs=3))
        epool = ctx.enter_context(tc.tile_pool(name="exp", bufs=3))
        opool = ctx.enter_context(tc.tile_pool(name="outp", bufs=3))
        ps_mm = ctx.enter_context(
            tc.tile_pool(name="ps_mm", bufs=2, space=bass.MemorySpace.PSUM)
        )
        ps_x2 = ctx.enter_context(
            tc.tile_pool(name="ps_x2", bufs=1, space=bass.MemorySpace.PSUM)
        )
        ps_pre = ctx.enter_context(
            tc.tile_pool(name="ps_pre", bufs=1, space=bass.MemorySpace.PSUM)
        )

        # warm-up weights: first gpsimd op so the PE warm-up can start
        # as early as possible
        warm_w = consts.tile([C, M], BF16)
        nc.gpsimd.memset(warm_w[:], 0.0)

        # ---- input DMAs (issued up front; Tile tracks readiness) ----
        # sync HWDGE ring: w, b first (needed earliest), then the
        # XBAR-transposed x group loads
        w_sb = consts.tile([C, M], F32)
        nc.sync.dma_start(w_sb[:], w_d[:])
        b_sb = consts.tile([1, M], F32)
        nc.sync.dma_start(b_sb[:], b_d[:])

        x_gv = x_d.rearrange("(g r) c -> g r c", r=GR)
        x_ts = []
        for g in range(N_G):
            x_t = xt_pool.tile([C, GR], BF16, tag="x_t")
            nc.sync.dma_start(x_t[:], x_gv[g], transpose=True)
            x_ts.append(x_t)

        # ---- one-time constants ----
        w_bf = consts.tile([C, M], BF16)
        nc.vector.tensor_copy(w_bf[:], w_sb[:])

        ones_c = consts.tile([C, 1], F32)
        nc.gpsimd.memset(ones_c[:], 1.0)
        neghalf_c = consts.tile([C, 1], BF16)
        nc.gpsimd.memset(neghalf_c[:], -0.5)
        ones_r_f = consts.tile([1, P], F32)
        nc.gpsimd.memset(ones_r_f[:], 1.0)

        # stationary aug tile: row 0 = -x2/2 (filled per-group by gpsimd
        # from the PE column-sum matmul), row 1 = ones.  Pool-engine
        # writes must start at partition 0, so the data row is row 0 and
        # the constant row comes from one [0:2] memset.
        stat2 = consts.tile([2, ROWS], BF16)
        nc.gpsimd.memset(stat2[0:2, :], 1.0)
        # moving aug tile: row 0 = ones, row 1 = -w2/2 (delivered into
        # partition 1 via SBUF->SBUF DMA below)
        rhs2 = consts.tile([2, M], BF16)
        nc.gpsimd.memset(rhs2[0:2, :], 1.0)

        # PE warm-up: dense back-to-back dummy matmuls so the HAM
        # clock-gate opens (1.2 -> 2.4 GHz) before the main stream.
        p_pre = ps_pre.tile([P, M], F32, tag="p_pre")
        for _ in range(12):
            nc.tensor.matmul(p_pre[:], warm_w[:, :P], warm_w[:], start=True,
                             stop=True)

        # w2[m] = sum_c w[c,m]^2 via ones.T @ (w*w); v = -w2/2, moved
        # into rhs2 row 1 by SBUF->SBUF DMA (any engine can write
        # partition 0, only DMA can land data at partition 1)
        wsq = consts.tile([C, M], F32)
        nc.vector.tensor_mul(wsq[:], w_sb[:], w_sb[:])
        p_w2 = ps_pre.tile([P, M], F32, tag="p_pre")
        nc.tensor.matmul(p_w2[:1, :], ones_c[:], wsq[:], start=True, stop=True)
        v_sb = consts.tile([1, M], BF16)
        nc.scalar.activation(v_sb[:], p_w2[:1, :], AF.Copy, scale=-0.5)
        nc.sync.dma_start(rhs2[1:2, :], v_sb[:])

        # bb3[p, j, m] = b[m] broadcast along partitions, G copies (bf16)
        p_bb = ps_pre.tile([P, M], F32, tag="p_pre")
        nc.tensor.matmul(p_bb[:], ones_r_f[:], b_sb[:], start=True, stop=True)
        bb3 = consts.tile([P, G, M], BF16)
        for j in range(G):
            nc.vector.tensor_copy(bb3[:, j, :], p_bb[:])

        # ---- main loop ----
        o_v = o_d.rearrange("(g j p) m -> g p j m", j=G, p=P)
        store_engs = [nc.sync, nc.scalar]

        xt2s = {}

        def emit_x2(g):
            """xt2 = x_t*x_t (gpsimd); -x2/2 row via PE; drain to stat2 (DVE).

            GPSIMD cannot touch PSUM, so the squares run there (SBUF
            only) and the small [1, GR] PSUM row drain goes to DVE."""
            xt2 = x2_pool.tile([C, GR], BF16, tag="xt2")
            nc.gpsimd.tensor_mul(xt2[:], x_ts[g][:], x_ts[g][:])
            px2 = ps_x2.tile([1, GR], F32, tag="px2")
            nc.tensor.matmul(px2[:], neghalf_c[:], xt2[:], start=True, stop=True)
            nc.vector.tensor_copy(stat2[0:1, g * GR : (g + 1) * GR], px2[:])

        emit_x2(0)
        emit_x2(1)

        pending = []

        for g in range(N_G):
            x_t = x_ts[g]
            p3 = ps_mm.tile([P, G, M], F32, tag="p3")
            for j in range(G):
                r0 = g * GR + j * P
                nc.tensor.matmul(
                    p3[:, j, :], stat2[0:2, r0 : r0 + P], rhs2[:],
                    start=True, stop=False,
                )
                nc.tensor.matmul(
                    p3[:, j, :],
                    x_t[:, j * P : (j + 1) * P],
                    w_bf[:],
                    start=False,
                    stop=True,
                )
            if g + 2 < N_G:
                emit_x2(g + 2)

            e3 = epool.tile([P, G, M], BF16, tag="e3")
            nc.scalar.activation(e3[:], p3[:], AF.Exp, scale=2.0)

            if g == N_G - 1:
                # tail: add+store in chunks so the final DMAs overlap the
                # adds, split across both HWDGE queues
                o_t = opool.tile([P, G, M], BF16, tag="o_t")
                nc.vector.tensor_add(o_t[:, :2, :], e3[:, :2, :], bb3[:, :2, :])
                nc.sync.dma_start(o_v[g][:, :2, :], o_t[:, :2, :])
                nc.vector.tensor_add(o_t[:, 2:, :], e3[:, 2:, :], bb3[:, 2:, :])
                nc.scalar.dma_start(o_v[g][:, 2:, :], o_t[:, 2:, :])
            else:
                o_t = opool.tile([P, G, M], BF16, tag="o_t")
                nc.vector.tensor_add(o_t[:], e3[:], bb3[:])
                pending.append((g, o_t))
                if len(pending) == 2 or g == N_G - 2:
                    eng = store_engs[(g // 2) % 2]
                    for gg, ot in pending:
                        eng.dma_start(o_v[gg], ot[:])
                    pending = []

    nc.compile()
    return nc


def _get_nc():
    if "nc" not in _NC_CACHE:
        _NC_CACHE["nc"] = _build_nc()
    return _NC_CACHE["nc"]


def _run(x, w, b, trace=False, tmpdir=None):
    nc = _get_nc()
    xs = (
        np.ascontiguousarray(np.asarray(x, dtype=np.float32))
        .reshape(N_CORES, ROWS, C)
        .astype(ml_dtypes.bfloat16)
    )
    wf = np.ascontiguousarray(np.asarray(w, dtype=np.float32))
    bf = np.ascontiguousarray(np.asarray(b, dtype=np.float32)).reshape(1, M)
    in_maps = [{"x": xs[i], "w": wf, "b": bf} for i in range(N_CORES)]
    res = run_bass_kernel_spmd(
        nc, in_maps, list(range(N_CORES)), trace=trace, tmpdir=tmpdir
    )
    out = np.stack([res.results[i]["out"] for i in range(N_CORES)], axis=0)
    return out.astype(np.float32).reshape(B, H * W_, M), res


def kernel(x, w, b):
    out, _ = _run(x, w, b, trace=False)
    return out


# revision 8
# speedup vs baseline: 1.1203x; 1.1203x over previous
"""Gaussian-kernel layer (exp(-||x - w_m||^2) + b_m) as a Bass/Tile TRN2 kernel.

Math (per row n of x, per center m):
    out[n, m] = exp(-(x2[n] + w2[m] - 2*x.w)) + b[m]
              = exp(2*(xw[n,m] - w2[m]/2 - x2[n]/2)) + b[m]

Mapping (v2.7):
  - x is cast to bf16 on the host and loaded ONLY via XBAR DMA-transpose
    (two row-groups per transfer, split across both HWDGE rings) ->
    x_t tiles [C, n].  No PE transpose-mode instructions: those don't
    count as PE-busy for the HAM clock gate and kept the v1 main loop
    at the cold 1.2 GHz clock.
  - x2 row-sums come from the transposed tiles with 4-group lookahead:
    xt2 = x_t*x_t (gpsimd, SBUF-only), a PE matmul against a [-0.5]
    column gives -x2/2 as a [1, n] PSUM row, and DVE/ACT drain it into
    row 0 of a persistent [2, ROWS] stationary tile whose row 1 is a
    host-fed ones row (Pool/DVE writes cannot start at partition 1).
  - K=2 preload: one matmul with lhsT=[-x2/2; ones], rhs=[ones; -w2/2]
    initializes PSUM with BOTH biases in a single 512-column stream,
    then the main bf16 matmul accumulates x.w on top.  rhs is built
    on-chip: selector rows [0,1]/[1,0] route -w2/2 and ones into a
    2-partition PSUM tile that ACT copies to SBUF.
  - ACT: bias-free exp(2*psum) over 3 PSUM banks per instruction.  The
    exp argument is the complete -d2 <= 0, so no overflow.
  - DVE: 3-tile-wide bf16 adds (+b), 2x packed mode.
  - Output stored as bf16 (rel tolerance is 2e-2; exp(-d2) <= 3e-44 on
    this distribution so the output is b to within fp32 epsilon and
    bf16 rounding is ~4e-3), widened to fp32 on the host.
"""

from contextlib import ExitStack

import numpy as np
import ml_dtypes

import concourse.bacc as bacc
import concourse.bass as bass
import concourse.mybir as mybir
import concourse.tile as tile
from concourse.bass_utils import run_bass_kernel_spmd

B, H, W_, C, M = 16, 48, 48, 128, 512
N_CORES = 8
B_PER = B // N_CORES          # 2 batches per core
ROWS = B_PER * H * W_         # 4608 rows per core
P = 128                       # partition / row-tile size
N_TILES = ROWS // P           # 36
G = 3                         # tiles per group
N_G = N_TILES // G            # 12
GR = G * P                    # rows per group (384)
LOOKAHEAD = 4                 # x2 pipeline lookahead in groups

F32 = mybir.dt.float32
BF16 = mybir.dt.bfloat16

_NC_CACHE = {}


def _build_nc():
    nc = bacc.Bacc(
        "TRN2",
        target_bir_lowering=False,
        debug=False,
        num_devices=N_CORES,
    )
    x_d = nc.declare_dram_parameter("x", [ROWS, C], BF16, isOutput=False)
    w_d = nc.declare_dram_parameter("w", [C, M], F32, isOutput=False)
    b_d = nc.declare_dram_parameter("b", [1, M], F32, isOutput=False)
    # [zeros-row; ones-row] constant: row 1 becomes the "ones" row of the
    # K=2 stationary tile (engines can't memset starting at partition 1)
    s2_d = nc.declare_dram_parameter("s2init", [2, ROWS], BF16, isOutput=False)
    o_d = nc.declare_dram_parameter("out", [ROWS, M], BF16, isOutput=True)

    AF = mybir.ActivationFunctionType

    with tile.TileContext(nc) as tc, ExitStack() as ctx:
        consts = ctx.enter_context(tc.tile_pool(name="consts", bufs=1))
        xt_pool = ctx.enter_context(tc.tile_pool(name="xt", bufs=N_G // 2))
        x2_pool = ctx.enter_context(tc.tile_pool(name="x2", bufs=3))
        epool = ctx.enter_context(tc.tile_pool(name="exp", bufs=3))
        opool = ctx.enter_context(tc.tile_pool(name="outp", bufs=3))
        ps_mm = ctx.enter_context(
            tc.tile_pool(name="ps_mm", bufs=2, space=bass.MemorySpace.PSUM)
        )
        ps_x2 = ctx.enter_context(
            tc.tile_pool(name="ps_x2", bufs=1, space=bass.MemorySpace.PSUM)
        )
        ps_pre = ctx.enter_context(
            tc.tile_pool(name="ps_pre", bufs=1, space=bass.MemorySpace.PSUM)
        )

        # warm-up weights on DVE (shortest prologue of the free engines)
        warm_w = consts.tile([C, M], BF16)
        nc.vector.memset(warm_w[:], 0.0)

        # ---- input DMAs (issued up front; Tile tracks readiness) ----
        w_sb = consts.tile([C, M], F32)
        nc.sync.dma_start(w_sb[:], w_d[:])
        b_sb = consts.tile([1, M], F32)
        nc.sync.dma_start(b_sb[:], b_d[:])

        # stationary aug tile: row 0 = -x2/2 (device-filled), row 1 = ones
        stat2 = consts.tile([2, ROWS], BF16)
        nc.scalar.dma_start(stat2[:], s2_d[:])

        # XBAR-transposed x loads: two groups per transfer, rings split
        x_gv = x_d.rearrange("(g r) c -> g r c", r=2 * GR)
        load_engs = [nc.sync, nc.scalar]
        x_t2s = []
        for g2 in range(N_G // 2):
            x_t2 = xt_pool.tile([C, 2 * GR], BF16, tag="x_t2")
            load_engs[g2 % 2].dma_start(x_t2[:], x_gv[g2], transpose=True)
            x_t2s.append(x_t2)

        def x_t(g):
            return x_t2s[g // 2][:, (g % 2) * GR : (g % 2 + 1) * GR]

        # ---- small constants (gpsimd, partition-0 only) ----
        ones_c = consts.tile([C, 1], F32)
        nc.gpsimd.memset(ones_c[:], 1.0)
        neghalf_c = consts.tile([C, 1], BF16)
        nc.gpsimd.memset(neghalf_c[:], -0.5)
        ones_r_f = consts.tile([1, P], F32)
        nc.gpsimd.memset(ones_r_f[:], 1.0)
        ones_m = consts.tile([1, M], BF16)
        nc.gpsimd.memset(ones_m[:], 1.0)
        # selector rows [0, 1] and [1, 0] for routing rows into PSUM
        colsel = consts.tile([1, 4], BF16)
        nc.gpsimd.memset(colsel[:, :], 0.0)
        nc.gpsimd.memset(colsel[:, 1:3], 1.0)
        sel01 = colsel[:, 0:2]  # [0, 1]
        sel10 = colsel[:, 2:4]  # [1, 0]

        # w*w and bf16 w on gpsimd (frees DVE/ACT for the pipeline)
        wsq = consts.tile([C, M], F32)
        nc.gpsimd.tensor_mul(wsq[:], w_sb[:], w_sb[:])
        w_bf = consts.tile([C, M], BF16)
        nc.gpsimd.tensor_copy(w_bf[:], w_sb[:])

        # ---- PE preamble, interleaved with warm-up so the HAM clock
        # gate opens right as the main stream starts ----
        p_pre = ps_pre.tile([P, M], F32, tag="p_pre")
        for _ in range(2):
            nc.tensor.matmul(p_pre[:], warm_w[:, :P], warm_w[:], start=True,
                             stop=True)
        # w2[m] = sum_c w[c,m]^2; v = -w2/2 via ACT
        p_w2 = ps_pre.tile([P, M], F32, tag="p_pre")
        nc.tensor.matmul(p_w2[:1, :], ones_c[:], wsq[:], start=True, stop=True)
        v_sb = consts.tile([1, M], BF16)
        nc.scalar.activation(v_sb[:], p_w2[:1, :], AF.Copy, scale=-0.5)
        for _ in range(3):
            nc.tensor.matmul(p_pre[:], warm_w[:, :P], warm_w[:], start=True,
                             stop=True)
        # rhs2 = [ones; -w2/2] assembled in PSUM via selector rows
        p_r2 = ps_pre.tile([P, M], F32, tag="p_pre")
        nc.tensor.matmul(p_r2[0:2, :], sel01, v_sb[:], start=True, stop=False)
        nc.tensor.matmul(p_r2[0:2, :], sel10, ones_m[:], start=False, stop=True)
        rhs2 = consts.tile([2, M], BF16)
        nc.scalar.activation(rhs2[:], p_r2[0:2, :], AF.Copy)
        # b broadcast along partitions; bb3 = G copies in bf16
        p_bb = ps_pre.tile([P, M], F32, tag="p_pre")
        nc.tensor.matmul(p_bb[:], ones_r_f[:], b_sb[:], start=True, stop=True)
        bb3 = consts.tile([P, G, M], BF16)
        nc.vector.tensor_copy(bb3[:, 0, :], p_bb[:])
        nc.vector.tensor_copy(bb3[:, 1, :], bb3[:, 0, :])
        nc.vector.tensor_copy(bb3[:, 2, :], bb3[:, 0, :])

        # ---- main loop ----
        o_v = o_d.rearrange("(g j p) m -> g p j m", j=G, p=P)
        store_engs = [nc.sync, nc.scalar]

        def emit_x2(g):
            """xt2 = x_t*x_t (gpsimd); -x2/2 row via PE; drain to stat2.

            GPSIMD cannot touch PSUM, so the [1, GR] PSUM row drain is
            split between DVE (8/12) and ACT (4/12)."""
            xt2 = x2_pool.tile([C, GR], BF16, tag="xt2")
            nc.gpsimd.tensor_mul(xt2[:], x_t(g), x_t(g))
            px2 = ps_x2.tile([1, GR], F32, tag="px2")
            nc.tensor.matmul(px2[:], neghalf_c[:], xt2[:], start=True, stop=True)
            dst = stat2[0:1, g * GR : (g + 1) * GR]
            if g % 3 == 2:
                nc.scalar.activation(dst, px2[:], AF.Copy)
            else:
                nc.vector.tensor_copy(dst, px2[:])

        for g in range(LOOKAHEAD):
            emit_x2(g)

        pending = []

        for g in range(N_G):
            p3 = ps_mm.tile([P, G, M], F32, tag="p3")
            for j in range(G):
                r0 = g * GR + j * P
                nc.tensor.matmul(
                    p3[:, j, :], stat2[0:2, r0 : r0 + P], rhs2[:],
                    start=True, stop=False,
                )
                nc.tensor.matmul(
                    p3[:, j, :],
                    x_t(g)[:, j * P : (j + 1) * P],
                    w_bf[:],
                    start=False,
                    stop=True,
                )
            if g + LOOKAHEAD < N_G:
                emit_x2(g + LOOKAHEAD)

            e3 = epool.tile([P, G, M], BF16, tag="e3")
            nc.scalar.activation(e3[:], p3[:], AF.Exp, scale=2.0)

            if g == N_G - 1:
                # tail: add+store in chunks so the final DMAs overlap the
                # adds, split across both HWDGE queues
                o_t = opool.tile([P, G, M], BF16, tag="o_t")
                nc.vector.tensor_add(o_t[:, :2, :], e3[:, :2, :], bb3[:, :2, :])
                nc.sync.dma_start(o_v[g][:, :2, :], o_t[:, :2, :])
                nc.vector.tensor_add(o_t[:, 2:, :], e3[:, 2:, :], bb3[:, 2:, :])
                nc.scalar.dma_start(o_v[g][:, 2:, :], o_t[:, 2:, :])
            else:
                o_t = opool.tile([P, G, M], BF16, tag="o_t")
                nc.vector.tensor_add(o_t[:], e3[:], bb3[:])
                pending.append((g, o_t))
                if len(pending) == 2 or g == N_G - 2:
                    eng = store_engs[(g // 2) % 2]
                    for gg, ot in pending:
                        eng.dma_start(o_v[gg], ot[:])
                    pending = []

    nc.compile()
    return nc


def _get_nc():
    if "nc" not in _NC_CACHE:
        _NC_CACHE["nc"] = _build_nc()
    return _NC_CACHE["nc"]


_S2INIT = np.concatenate(
    [np.zeros((1, ROWS)), np.ones((1, ROWS))], axis=0
).astype(ml_dtypes.bfloat16)


def _run(x, w, b, trace=False, tmpdir=None):
    nc = _get_nc()
    xs = (
        np.ascontiguousarray(np.asarray(x, dtype=np.float32))
        .reshape(N_CORES, ROWS, C)
        .astype(ml_dtypes.bfloat16)
    )
    wf = np.ascontiguousarray(np.asarray(w, dtype=np.float32))
    bf = np.ascontiguousarray(np.asarray(b, dtype=np.float32)).reshape(1, M)
    in_maps = [
        {"x": xs[i], "w": wf, "b": bf, "s2init": _S2INIT} for i in range(N_CORES)
    ]
    res = run_bass_kernel_spmd(
        nc, in_maps, list(range(N_CORES)), trace=trace, tmpdir=tmpdir
    )
    out = np.stack([res.results[i]["out"] for i in range(N_CORES)], axis=0)
    return out.astype(np.float32).reshape(B, H * W_, M), res


def kernel(x, w, b):
    out, _ = _run(x, w, b, trace=False)
    return out


# revision 11
# speedup vs baseline: 1.1613x; 1.0366x over previous
"""Gaussian-kernel layer (exp(-||x - w_m||^2) + b_m) as a Bass/Tile TRN2 kernel.

Math (per row n of x, per center m):
    out[n, m] = exp(-(x2[n] + w2[m] - 2*x.w)) + b[m]
              = exp(2*(xw[n,m] - w2[m]/2 - x2[n]/2)) + b[m]

Mapping (v2.7):
  - x is cast to bf16 on the host and loaded ONLY via XBAR DMA-transpose
    (two row-groups per transfer, split across both HWDGE rings) ->
    x_t tiles [C, n].  No PE transpose-mode instructions: those don't
    count as PE-busy for the HAM clock gate and kept the v1 main loop
    at the cold 1.2 GHz clock.
  - x2 row-sums come from the transposed tiles with 4-group lookahead:
    xt2 = x_t*x_t (gpsimd, SBUF-only), a PE matmul against a [-0.5]
    column gives -x2/2 as a [1, n] PSUM row, and DVE/ACT drain it into
    row 0 of a persistent [2, ROWS] stationary tile whose row 1 is a
    host-fed ones row (Pool/DVE writes cannot start at partition 1).
  - K=2 preload: one matmul with lhsT=[-x2/2; ones], rhs=[ones; -w2/2]
    initializes PSUM with BOTH biases in a single 512-column stream,
    then the main bf16 matmul accumulates x.w on top.  rhs is built
    on-chip: selector rows [0,1]/[1,0] route -w2/2 and ones into a
    2-partition PSUM tile that ACT copies to SBUF.
  - ACT: bias-free exp(2*psum) over 3 PSUM banks per instruction.  The
    exp argument is the complete -d2 <= 0, so no overflow.
  - DVE: 3-tile-wide bf16 adds (+b), 2x packed mode.
  - Output stored as bf16 (rel tolerance is 2e-2; exp(-d2) <= 3e-44 on
    this distribution so the output is b to within fp32 epsilon and
    bf16 rounding is ~4e-3), widened to fp32 on the host.
"""

from contextlib import ExitStack

import numpy as np
import ml_dtypes

import concourse.bacc as bacc
import concourse.bass as bass
import concourse.mybir as mybir
import concourse.tile as tile
from concourse.bass_utils import run_bass_kernel_spmd

B, H, W_, C, M = 16, 48, 48, 128, 512
N_CORES = 8
B_PER = B // N_CORES          # 2 batches per core
ROWS = B_PER * H * W_         # 4608 rows per core
P = 128                       # partition / row-tile size
N_TILES = ROWS // P           # 36
G = 3                         # tiles per group
N_G = N_TILES // G            # 12
GR = G * P                    # rows per group (384)
LOOKAHEAD = 4                 # x2 pipeline lookahead in groups

F32 = mybir.dt.float32
BF16 = mybir.dt.bfloat16

_NC_CACHE = {}


def _build_nc():
    nc = bacc.Bacc(
        "TRN2",
        target_bir_lowering=False,
        debug=False,
        num_devices=N_CORES,
    )
    x_d = nc.declare_dram_parameter("x", [ROWS, C], BF16, isOutput=False)
    w_d = nc.declare_dram_parameter("w", [C, M], F32, isOutput=False)
    b_d = nc.declare_dram_parameter("b", [1, M], F32, isOutput=False)
    # [zeros-row; ones-row] constant: row 1 becomes the "ones" row of the
    # K=2 stationary tile (engines can't memset starting at partition 1)
    s2_d = nc.declare_dram_parameter("s2init", [2, ROWS], BF16, isOutput=False)
    o_d = nc.declare_dram_parameter("out", [ROWS, M], BF16, isOutput=True)

    AF = mybir.ActivationFunctionType

    with tile.TileContext(nc) as tc, ExitStack() as ctx:
        consts = ctx.enter_context(tc.tile_pool(name="consts", bufs=1))
        xt_pool = ctx.enter_context(tc.tile_pool(name="xt", bufs=N_G // 2))
        x2_pool = ctx.enter_context(tc.tile_pool(name="x2", bufs=3))
        epool = ctx.enter_context(tc.tile_pool(name="exp", bufs=3))
        opool = ctx.enter_context(tc.tile_pool(name="outp", bufs=3))
        ps_mm = ctx.enter_context(
            tc.tile_pool(name="ps_mm", bufs=2, space=bass.MemorySpace.PSUM)
        )
        # shared 2-bank pool for preamble tiles and the phase-0 -x2/2
        # row matmuls (8-bank budget: 2*3 main + these 2)
        ps_aux = ctx.enter_context(
            tc.tile_pool(name="ps_aux", bufs=2, space=bass.MemorySpace.PSUM)
        )

        # warm-up weights on DVE (shortest prologue of the free engines)
        warm_w = consts.tile([C, M], BF16)
        nc.vector.memset(warm_w[:], 0.0)

        # ---- input DMAs (issued up front; Tile tracks readiness) ----
        # w first on sync (it heads the -w2/2 dependency chain), then
        # the XBAR-transposed x loads (two groups per transfer, rings
        # split; the XBAR unit is globally serialized at ~1.3us/192KB so
        # they must start as early as possible).
        w_sb = consts.tile([C, M], F32)
        nc.sync.dma_start(w_sb[:], w_d[:])
        b_sb = consts.tile([1, M], F32)
        nc.sync.dma_start(b_sb[:], b_d[:])

        x_gv = x_d.rearrange("(g r) c -> g r c", r=2 * GR)
        load_engs = [nc.sync, nc.scalar]
        x_t2s = []
        for g2 in range(N_G // 2):
            x_t2 = xt_pool.tile([C, 2 * GR], BF16, tag="x_t2")
            load_engs[g2 % 2].dma_start(x_t2[:], x_gv[g2], transpose=True)
            x_t2s.append(x_t2)

        # stationary aug tile: row 0 = -x2/2 (device-filled), row 1 = ones
        stat2 = consts.tile([2, ROWS], BF16)
        nc.scalar.dma_start(stat2[:], s2_d[:])

        def x_t(g):
            return x_t2s[g // 2][:, (g % 2) * GR : (g % 2 + 1) * GR]

        # ---- small constants (gpsimd, partition-0 only) ----
        ones_c = consts.tile([C, 1], F32)
        nc.gpsimd.memset(ones_c[:], 1.0)
        neghalf_c = consts.tile([C, 1], BF16)
        nc.gpsimd.memset(neghalf_c[:], -0.5)
        ones_r_f = consts.tile([1, P], F32)
        nc.gpsimd.memset(ones_r_f[:], 1.0)
        ones_m = consts.tile([1, M], BF16)
        nc.gpsimd.memset(ones_m[:], 1.0)
        # selector rows [0, 1] and [1, 0] for routing rows into PSUM
        colsel = consts.tile([1, 4], BF16)
        nc.gpsimd.memset(colsel[:, :], 0.0)
        nc.gpsimd.memset(colsel[:, 1:3], 1.0)
        sel01 = colsel[:, 0:2]  # [0, 1]
        sel10 = colsel[:, 2:4]  # [1, 0]

        # w*w and bf16 w on gpsimd (frees DVE/ACT for the pipeline)
        wsq = consts.tile([C, M], F32)
        nc.gpsimd.tensor_mul(wsq[:], w_sb[:], w_sb[:])
        w_bf = consts.tile([C, M], BF16)
        nc.gpsimd.tensor_copy(w_bf[:], w_sb[:])

        # ---- PE preamble + phase-0 x2 pipeline ----
        # All cross-engine x2 work happens BEFORE the matmul stream so
        # the stream itself is hole-free: any sub-400ns stall per group
        # keeps the HAM SHORT window from ever seeing a contiguous
        # 3.4us of busy, locking the PE at 1.2 GHz.
        p_pre = ps_aux.tile([P, M], F32, tag="p_aux")
        for _ in range(2):
            nc.tensor.matmul(p_pre[:], warm_w[:, :P], warm_w[:], start=True,
                             stop=True)
        # w2[m] = sum_c w[c,m]^2; v = -w2/2 via ACT
        p_w2 = ps_aux.tile([P, M], F32, tag="p_aux")
        nc.tensor.matmul(p_w2[:1, :], ones_c[:], wsq[:], start=True, stop=True)
        v_sb = consts.tile([1, M], BF16)
        nc.scalar.activation(v_sb[:], p_w2[:1, :], AF.Copy, scale=-0.5)
        # rhs2 = [ones; -w2/2] assembled in PSUM via selector rows
        p_r2 = ps_aux.tile([P, M], F32, tag="p_aux")
        nc.tensor.matmul(p_r2[0:2, :], sel01, v_sb[:], start=True, stop=False)
        nc.tensor.matmul(p_r2[0:2, :], sel10, ones_m[:], start=False, stop=True)
        rhs2 = consts.tile([2, M], BF16)
        nc.scalar.activation(rhs2[:], p_r2[0:2, :], AF.Copy)
        # b broadcast along partitions; bb3 = G copies in bf16
        p_bb = ps_aux.tile([P, M], F32, tag="p_aux")
        nc.tensor.matmul(p_bb[:], ones_r_f[:], b_sb[:], start=True, stop=True)
        bb3 = consts.tile([P, G, M], BF16)
        nc.vector.tensor_copy(bb3[:, 0, :], p_bb[:])
        nc.vector.tensor_copy(bb3[:, 1, :], bb3[:, 0, :])
        nc.vector.tensor_copy(bb3[:, 2, :], bb3[:, 0, :])

        def emit_x2(g):
            """xt2 = x_t*x_t (gpsimd); -x2/2 row via PE; drain to stat2.

            GPSIMD cannot touch PSUM, so the [1, GR] PSUM row drain is
            split between DVE (8/12) and ACT (4/12)."""
            xt2 = x2_pool.tile([C, GR], BF16, tag="xt2")
            nc.gpsimd.tensor_mul(xt2[:], x_t(g), x_t(g))
            px2 = ps_aux.tile([1, GR], F32, tag="p_aux")
            nc.tensor.matmul(px2[:], neghalf_c[:], xt2[:], start=True, stop=True)
            dst = stat2[0:1, g * GR : (g + 1) * GR]
            if g % 3 == 2:
                nc.scalar.activation(dst, px2[:], AF.Copy)
            else:
                nc.vector.tensor_copy(dst, px2[:])

        for g in range(N_G):
            emit_x2(g)

        # ---- main loop: a pure, dense preload+matmul stream ----
        o_v = o_d.rearrange("(g j p) m -> g p j m", j=G, p=P)
        store_engs = [nc.sync, nc.scalar]

        pending = []

        for g in range(N_G):
            p3 = ps_mm.tile([P, G, M], F32, tag="p3")
            for j in range(G):
                r0 = g * GR + j * P
                nc.tensor.matmul(
                    p3[:, j, :], stat2[0:2, r0 : r0 + P], rhs2[:],
                    start=True, stop=False,
                )
                nc.tensor.matmul(
                    p3[:, j, :],
                    x_t(g)[:, j * P : (j + 1) * P],
                    w_bf[:],
                    start=False,
                    stop=True,
                )

            if g == N_G - 1:
                # tail: per-tile exp+add+store so the final chain is as
                # short as possible, stores split across both rings
                o_t = opool.tile([P, G, M], BF16, tag="o_t")
                e3 = epool.tile([P, G, M], BF16, tag="e3")
                for j in range(G):
                    nc.scalar.activation(
                        e3[:, j, :], p3[:, j, :], AF.Exp, scale=2.0
                    )
                    nc.vector.tensor_add(
                        o_t[:, j, :], e3[:, j, :], bb3[:, j, :]
                    )
                    store_engs[j % 2].dma_start(o_v[g][:, j, :], o_t[:, j, :])
            else:
                e3 = epool.tile([P, G, M], BF16, tag="e3")
                nc.scalar.activation(e3[:], p3[:], AF.Exp, scale=2.0)
                o_t = opool.tile([P, G, M], BF16, tag="o_t")
                nc.vector.tensor_add(o_t[:], e3[:], bb3[:])
                pending.append((g, o_t))
                if len(pending) == 2 or g == N_G - 2:
                    eng = store_engs[(g // 2) % 2]
                    for gg, ot in pending:
                        eng.dma_start(o_v[gg], ot[:])
                    pending = []

    nc.compile()
    return nc


def _get_nc():
    if "nc" not in _NC_CACHE:
        _NC_CACHE["nc"] = _build_nc()
    return _NC_CACHE["nc"]


_S2INIT = np.concatenate(
    [np.zeros((1, ROWS)), np.ones((1, ROWS))], axis=0
).astype(ml_dtypes.bfloat16)


def _run(x, w, b, trace=False, tmpdir=None):
    nc = _get_nc()
    xs = (
        np.ascontiguousarray(np.asarray(x, dtype=np.float32))
        .reshape(N_CORES, ROWS, C)
        .astype(ml_dtypes.bfloat16)
    )
    wf = np.ascontiguousarray(np.asarray(w, dtype=np.float32))
    bf = np.ascontiguousarray(np.asarray(b, dtype=np.float32)).reshape(1, M)
    in_maps = [
        {"x": xs[i], "w": wf, "b": bf, "s2init": _S2INIT} for i in range(N_CORES)
    ]
    res = run_bass_kernel_spmd(
        nc, in_maps, list(range(N_CORES)), trace=trace, tmpdir=tmpdir
    )
    out = np.stack([res.results[i]["out"] for i in range(N_CORES)], axis=0)
    return out.astype(np.float32).reshape(B, H * W_, M), res


def kernel(x, w, b):
    out, _ = _run(x, w, b, trace=False)
    return out


# revision 12
# speedup vs baseline: 1.4075x; 1.2120x over previous
"""Gaussian-kernel layer (exp(-||x - w_m||^2) + b_m) as a Bass/Tile TRN2 kernel.

Math (per row n of x, per center m):
    out[n, m] = exp(-(x2[n] + w2[m] - 2*x.w)) + b[m]
              = exp(2*(xw[n,m] - w2[m]/2 - x2[n]/2)) + b[m]

Mapping (v2.7):
  - x is cast to bf16 on the host and loaded ONLY via XBAR DMA-transpose
    (two row-groups per transfer, split across both HWDGE rings) ->
    x_t tiles [C, n].  No PE transpose-mode instructions: those don't
    count as PE-busy for the HAM clock gate and kept the v1 main loop
    at the cold 1.2 GHz clock.
  - x2 row-sums come from the transposed tiles with 4-group lookahead:
    xt2 = x_t*x_t (gpsimd, SBUF-only), a PE matmul against a [-0.5]
    column gives -x2/2 as a [1, n] PSUM row, and DVE/ACT drain it into
    row 0 of a persistent [2, ROWS] stationary tile whose row 1 is a
    host-fed ones row (Pool/DVE writes cannot start at partition 1).
  - K=2 preload: one matmul with lhsT=[-x2/2; ones], rhs=[ones; -w2/2]
    initializes PSUM with BOTH biases in a single 512-column stream,
    then the main bf16 matmul accumulates x.w on top.  rhs is built
    on-chip: selector rows [0,1]/[1,0] route -w2/2 and ones into a
    2-partition PSUM tile that ACT copies to SBUF.
  - ACT: bias-free exp(2*psum) over 3 PSUM banks per instruction.  The
    exp argument is the complete -d2 <= 0, so no overflow.
  - DVE: 3-tile-wide bf16 adds (+b), 2x packed mode.
  - Output stored as bf16 (rel tolerance is 2e-2; exp(-d2) <= 3e-44 on
    this distribution so the output is b to within fp32 epsilon and
    bf16 rounding is ~4e-3), widened to fp32 on the host.
"""

from contextlib import ExitStack

import numpy as np
import ml_dtypes

import concourse.bacc as bacc
import concourse.bass as bass
import concourse.mybir as mybir
import concourse.tile as tile
from concourse.bass_utils import run_bass_kernel_spmd

B, H, W_, C, M = 16, 48, 48, 128, 512
N_CORES = 8
B_PER = B // N_CORES          # 2 batches per core
ROWS = B_PER * H * W_         # 4608 rows per core
P = 128                       # partition / row-tile size
N_TILES = ROWS // P           # 36
G = 3                         # tiles per group
N_G = N_TILES // G            # 12
GR = G * P                    # rows per group (384)
LOOKAHEAD = 4                 # x2 pipeline lookahead in groups

F32 = mybir.dt.float32
BF16 = mybir.dt.bfloat16

_NC_CACHE = {}


def _build_nc():
    nc = bacc.Bacc(
        "TRN2",
        target_bir_lowering=False,
        debug=False,
        num_devices=N_CORES,
    )
    x_d = nc.declare_dram_parameter("x", [ROWS, C], BF16, isOutput=False)
    w_d = nc.declare_dram_parameter("w", [C, M], F32, isOutput=False)
    b_d = nc.declare_dram_parameter("b", [1, M], F32, isOutput=False)
    # [zeros-row; ones-row] constant: row 1 becomes the "ones" row of the
    # K=2 stationary tile (engines can't memset starting at partition 1)
    s2_d = nc.declare_dram_parameter("s2init", [2, ROWS], BF16, isOutput=False)
    o_d = nc.declare_dram_parameter("out", [ROWS, M], BF16, isOutput=True)

    AF = mybir.ActivationFunctionType

    with tile.TileContext(nc) as tc, ExitStack() as ctx:
        consts = ctx.enter_context(tc.tile_pool(name="consts", bufs=1))
        xt_pool = ctx.enter_context(tc.tile_pool(name="xt", bufs=N_G // 2))
        x2_pool = ctx.enter_context(tc.tile_pool(name="x2", bufs=3))
        epool = ctx.enter_context(tc.tile_pool(name="exp", bufs=3))
        opool = ctx.enter_context(tc.tile_pool(name="outp", bufs=3))
        ps_mm = ctx.enter_context(
            tc.tile_pool(name="ps_mm", bufs=2, space=bass.MemorySpace.PSUM)
        )
        # shared 2-bank pool for preamble tiles and the phase-0 -x2/2
        # row matmuls (8-bank budget: 2*3 main + these 2)
        ps_aux = ctx.enter_context(
            tc.tile_pool(name="ps_aux", bufs=2, space=bass.MemorySpace.PSUM)
        )

        # warm-up weights on DVE (shortest prologue of the free engines)
        warm_w = consts.tile([C, M], BF16)
        nc.vector.memset(warm_w[:], 0.0)

        # ---- input DMAs (issued up front; Tile tracks readiness) ----
        # w first on sync (it heads the -w2/2 dependency chain), then
        # the XBAR-transposed x loads (two groups per transfer, rings
        # split; the XBAR unit is globally serialized at ~1.3us/192KB so
        # they must start as early as possible).
        w_sb = consts.tile([C, M], F32)
        nc.sync.dma_start(w_sb[:], w_d[:])
        b_sb = consts.tile([1, M], F32)
        nc.sync.dma_start(b_sb[:], b_d[:])

        x_gv = x_d.rearrange("(g r) c -> g r c", r=2 * GR)
        load_engs = [nc.sync, nc.scalar]
        x_t2s = []
        for g2 in range(N_G // 2):
            x_t2 = xt_pool.tile([C, 2 * GR], BF16, tag="x_t2")
            load_engs[g2 % 2].dma_start(x_t2[:], x_gv[g2], transpose=True)
            x_t2s.append(x_t2)

        # stationary aug tile: row 0 = -x2/2 (device-filled), row 1 =
        # ones (host-fed), rows 2..127 = 0.  Full-K padding: the HAM
        # busy detector appears to track PE array utilization, so K=1/2
        # preload matmuls read as idle and lock the clock at 1.2 GHz —
        # a K=128 preload with zero rows counts as busy.
        stat2 = consts.tile([C, ROWS], BF16)
        nc.vector.memset(stat2[:], 0.0)
        nc.scalar.dma_start(stat2[0:2, :], s2_d[:])

        def x_t(g):
            return x_t2s[g // 2][:, (g % 2) * GR : (g % 2 + 1) * GR]

        # ---- small constants (gpsimd, partition-0 only) ----
        ones_c = consts.tile([C, 1], F32)
        nc.gpsimd.memset(ones_c[:], 1.0)
        neghalf_c = consts.tile([C, 1], BF16)
        nc.gpsimd.memset(neghalf_c[:], -0.5)
        ones_r_f = consts.tile([1, P], F32)
        nc.gpsimd.memset(ones_r_f[:], 1.0)
        ones_m = consts.tile([1, M], BF16)
        nc.gpsimd.memset(ones_m[:], 1.0)
        # selector rows [0, 1] and [1, 0] for routing rows into PSUM
        colsel = consts.tile([1, 4], BF16)
        nc.gpsimd.memset(colsel[:, :], 0.0)
        nc.gpsimd.memset(colsel[:, 1:3], 1.0)
        sel01 = colsel[:, 0:2]  # [0, 1]
        sel10 = colsel[:, 2:4]  # [1, 0]

        # w*w and bf16 w on gpsimd (frees DVE/ACT for the pipeline)
        wsq = consts.tile([C, M], F32)
        nc.gpsimd.tensor_mul(wsq[:], w_sb[:], w_sb[:])
        w_bf = consts.tile([C, M], BF16)
        nc.gpsimd.tensor_copy(w_bf[:], w_sb[:])

        # ---- PE preamble + phase-0 x2 pipeline ----
        # All cross-engine x2 work happens BEFORE the matmul stream so
        # the stream itself is hole-free: any sub-400ns stall per group
        # keeps the HAM SHORT window from ever seeing a contiguous
        # 3.4us of busy, locking the PE at 1.2 GHz.
        p_pre = ps_aux.tile([P, M], F32, tag="p_aux")
        for _ in range(3):
            nc.tensor.matmul(p_pre[:], warm_w[:, :P], warm_w[:], start=True,
                             stop=True)
        # w2[m] = sum_c w[c,m]^2; v = -w2/2 via ACT
        p_w2 = ps_aux.tile([P, M], F32, tag="p_aux")
        nc.tensor.matmul(p_w2[:1, :], ones_c[:], wsq[:], start=True, stop=True)
        v_sb = consts.tile([1, M], BF16)
        nc.scalar.activation(v_sb[:], p_w2[:1, :], AF.Copy, scale=-0.5)
        p_pre2 = ps_aux.tile([P, M], F32, tag="p_aux")
        for _ in range(4):
            nc.tensor.matmul(p_pre2[:], warm_w[:, :P], warm_w[:], start=True,
                             stop=True)
        # rhs2 rows 0:2 = [ones; -w2/2] assembled in PSUM via selector
        # rows; rows 2..127 = 0 (full-K padding, see stat2 note)
        p_r2 = ps_aux.tile([P, M], F32, tag="p_aux")
        nc.tensor.matmul(p_r2[0:2, :], sel01, v_sb[:], start=True, stop=False)
        nc.tensor.matmul(p_r2[0:2, :], sel10, ones_m[:], start=False, stop=True)
        rhs2 = consts.tile([C, M], BF16)
        nc.vector.memset(rhs2[:], 0.0)
        nc.scalar.activation(rhs2[0:2, :], p_r2[0:2, :], AF.Copy)
        # b broadcast along partitions; bb3 = G copies in bf16
        p_bb = ps_aux.tile([P, M], F32, tag="p_aux")
        nc.tensor.matmul(p_bb[:], ones_r_f[:], b_sb[:], start=True, stop=True)
        bb3 = consts.tile([P, G, M], BF16)
        nc.vector.tensor_copy(bb3[:, 0, :], p_bb[:])
        nc.vector.tensor_copy(bb3[:, 1, :], bb3[:, 0, :])
        nc.vector.tensor_copy(bb3[:, 2, :], bb3[:, 0, :])

        def emit_x2(g):
            """xt2 = x_t*x_t (gpsimd); -x2/2 row via PE; drain to stat2.

            GPSIMD cannot touch PSUM, so the [1, GR] PSUM row drain is
            split between DVE (8/12) and ACT (4/12)."""
            xt2 = x2_pool.tile([C, GR], BF16, tag="xt2")
            nc.gpsimd.tensor_mul(xt2[:], x_t(g), x_t(g))
            px2 = ps_aux.tile([1, GR], F32, tag="p_aux")
            nc.tensor.matmul(px2[:], neghalf_c[:], xt2[:], start=True, stop=True)
            dst = stat2[0:1, g * GR : (g + 1) * GR]
            if g % 3 == 2:
                nc.scalar.activation(dst, px2[:], AF.Copy)
            else:
                nc.vector.tensor_copy(dst, px2[:])

        for g in range(N_G):
            emit_x2(g)

        # ---- main loop: a pure, dense preload+matmul stream ----
        o_v = o_d.rearrange("(g j p) m -> g p j m", j=G, p=P)
        store_engs = [nc.sync, nc.scalar]

        pending = []

        for g in range(N_G):
            p3 = ps_mm.tile([P, G, M], F32, tag="p3")
            for j in range(G):
                r0 = g * GR + j * P
                nc.tensor.matmul(
                    p3[:, j, :], stat2[:, r0 : r0 + P], rhs2[:],
                    start=True, stop=False,
                )
                nc.tensor.matmul(
                    p3[:, j, :],
                    x_t(g)[:, j * P : (j + 1) * P],
                    w_bf[:],
                    start=False,
                    stop=True,
                )

            if g == N_G - 1:
                # tail: per-tile exp+add+store so the final chain is as
                # short as possible, stores split across both rings
                o_t = opool.tile([P, G, M], BF16, tag="o_t")
                e3 = epool.tile([P, G, M], BF16, tag="e3")
                for j in range(G):
                    nc.scalar.activation(
                        e3[:, j, :], p3[:, j, :], AF.Exp, scale=2.0
                    )
                    nc.vector.tensor_add(
                        o_t[:, j, :], e3[:, j, :], bb3[:, j, :]
                    )
                    store_engs[j % 2].dma_start(o_v[g][:, j, :], o_t[:, j, :])
            else:
                e3 = epool.tile([P, G, M], BF16, tag="e3")
                nc.scalar.activation(e3[:], p3[:], AF.Exp, scale=2.0)
                o_t = opool.tile([P, G, M], BF16, tag="o_t")
                nc.vector.tensor_add(o_t[:], e3[:], bb3[:])
                pending.append((g, o_t))
                if len(pending) == 2 or g == N_G - 2:
                    eng = store_engs[(g // 2) % 2]
                    for gg, ot in pending:
                        eng.dma_start(o_v[gg], ot[:])
                    pending = []

    nc.compile()
    return nc


def _get_nc():
    if "nc" not in _NC_CACHE:
        _NC_CACHE["nc"] = _build_nc()
    return _NC_CACHE["nc"]


_S2INIT = np.concatenate(
    [np.zeros((1, ROWS)), np.ones((1, ROWS))], axis=0
).astype(ml_dtypes.bfloat16)


def _run(x, w, b, trace=False, tmpdir=None):
    nc = _get_nc()
    xs = (
        np.ascontiguousarray(np.asarray(x, dtype=np.float32))
        .reshape(N_CORES, ROWS, C)
        .astype(ml_dtypes.bfloat16)
    )
    wf = np.ascontiguousarray(np.asarray(w, dtype=np.float32))
    bf = np.ascontiguousarray(np.asarray(b, dtype=np.float32)).reshape(1, M)
    in_maps = [
        {"x": xs[i], "w": wf, "b": bf, "s2init": _S2INIT} for i in range(N_CORES)
    ]
    res = run_bass_kernel_spmd(
        nc, in_maps, list(range(N_CORES)), trace=trace, tmpdir=tmpdir
    )
    out = np.stack([res.results[i]["out"] for i in range(N_CORES)], axis=0)
    return out.astype(np.float32).reshape(B, H * W_, M), res


def kernel(x, w, b):
    out, _ = _run(x, w, b, trace=False)
    return out
